# revision 21
# baseline (speedup 1.0000x reference)
"""Causal GQA attention block (B=2, T=2048, C=2048, H=16, HKV=4, D=128, RoPE)
on 8 Trainium2 NeuronCores.

Sharding: core c handles batch b = c//4 and kv-group g = c%4 (4 q heads +
1 kv head per core).  The output projection is row-parallel: each core
produces a partial [T, C] contribution; the host sums the 4 partials per
batch.

v2 design (single fused jt-pipelined loop, fp16 storage):
  - all SBUF tensors fp16 (magnitudes here stay < 1e4, fp16 keeps per-elem
    rel err ~5e-4; matmuls run at 1 cycle/row like bf16).
  - x is host-transposed to xT [C, T]; q/k projections produce [D, T]
    head-transposed tiles; RoPE applied via host-side even/odd permutation
    folded into wq/wk + partition-swapped multiplies.
  - v is produced directly in [T, D] layout (x chunk as the stationary
    operand, wv as moving) -- no PE transposes at all.
  - scores are computed transposed (S.T tiles [s, t]); causal structure is
    exploited at fine grain: diagonal s-tiles only compute the t >= s part
    (moving dim shortened to 512-128r), with a single [128,128] triangular
    multiplicative mask for the crossing block.
  - softmax: no max-subtraction (scores are O(5), exp safe in fp32 PSUM);
    1/sqrt(D) folded into the exp's scale argument (free on ACT).
    The denominator is accumulated on DVE (fp16 adds of exp'd tiles) and
    partition-reduced+broadcast in one gpsimd partition_all_reduce -- no
    tensor-engine work.
  - the output projection for chunk jt-1 is interleaved into the attention
    h-loop of chunk jt (one tt row-block per head) so its matmuls fill the
    attention phase's dependency bubbles; it reuses the scores PSUM ring.
  - x chunks stream in 4-contraction-tile DMAs (few, large transfers --
    the cost model serializes descriptor generation per DMA instruction).
"""

import os
from contextlib import ExitStack

import numpy as np

import concourse.bass as bass
import concourse.tile as tile
from concourse import bass_isa
from concourse import bacc, mybir
from concourse.bass_utils import run_bass_kernel_spmd

# problem constants
B, T, C = 2, 2048, 2048
H, HKV, D = 16, 4, 128
GROUP = H // HKV           # 4 q heads per kv head
THETA = 1000000.0
SCALE = D ** -0.5

P = 128                    # partitions
TCH = 512                  # t-chunk (matmul moving free dim)
NJT = T // TCH             # 4 t-chunks
NK = C // P                # 16 contraction tiles
NH = GROUP                 # 4 local q heads
N_CORES = 8

F32 = mybir.dt.float32
AF = mybir.ActivationFunctionType
ALU = mybir.AluOpType


def _sb_dt(mode):
    return {"f16": mybir.dt.float16, "bf16": mybir.dt.bfloat16}[mode]


def _np_dt(mode):
    if mode == "bf16":
        import ml_dtypes
        return ml_dtypes.bfloat16
    return np.float16


def build_program(mode="f16", phases="ABC", variant=""):
    """Build and compile the per-core Bass program. Returns nc."""
    sb_dt = _sb_dt(mode)

    nc = bacc.Bacc("TRN2", target_bir_lowering=False, debug=False)

    xT_d = nc.dram_tensor("xT", [C, T], sb_dt, kind="ExternalInput").ap()
    wq_d = nc.dram_tensor("wqT", [C, NH * D], sb_dt, kind="ExternalInput").ap()
    wk_d = nc.dram_tensor("wkT", [P, NK, D], sb_dt, kind="ExternalInput").ap()
    wv_d = nc.dram_tensor("wvT", [P, NK, D], sb_dt, kind="ExternalInput").ap()
    wo_d = nc.dram_tensor("woT", [NH * D, C], sb_dt, kind="ExternalInput").ap()
    cos_d = nc.dram_tensor("cosT", [P, T], sb_dt, kind="ExternalInput").ap()
    sin_d = nc.dram_tensor("sinT", [P, T], sb_dt, kind="ExternalInput").ap()
    tri_d = nc.dram_tensor("triT", [P, P], sb_dt, kind="ExternalInput").ap()
    y_d = nc.dram_tensor("y", [T, C], sb_dt, kind="ExternalOutput").ap()

    with tile.TileContext(nc) as tc, ExitStack() as ctx:
        wpool = ctx.enter_context(tc.tile_pool(name="weights", bufs=1))
        tpool = ctx.enter_context(tc.tile_pool(name="tables", bufs=1))
        state = ctx.enter_context(tc.tile_pool(name="state", bufs=1))
        xpool = ctx.enter_context(tc.tile_pool(name="xsub", bufs=2))
        qkp = ctx.enter_context(tc.tile_pool(name="qkstage", bufs=3))
        ropep = ctx.enter_context(tc.tile_pool(name="rope", bufs=2))
        esp = ctx.enter_context(tc.tile_pool(name="es", bufs=8))
        dnp = ctx.enter_context(tc.tile_pool(name="dn", bufs=2))
        ysp = ctx.enter_context(tc.tile_pool(name="ys", bufs=4))
        psA = ctx.enter_context(tc.tile_pool(name="psA", bufs=2, space="PSUM"))
        psS = ctx.enter_context(tc.tile_pool(name="psS", bufs=2, space="PSUM"))
        psO = ctx.enter_context(tc.tile_pool(name="psO", bufs=2, space="PSUM"))

        # ---- weight / table loads -------------------------------------
        wq_sb = wpool.tile([P, NK, NH * D], sb_dt, tag="wq")
        wk_sb = wpool.tile([P, NK, D], sb_dt, tag="wk")
        wv_sb = wpool.tile([P, NK, D], sb_dt, tag="wv")
        def load_wq(k0, k1, eng):
            eng.dma_start(
                wq_sb[:, k0:k1, :],
                wq_d[k0 * P:k1 * P, :].rearrange("(ko p) o -> p ko o", p=P))
        # nudge wk's SWDGE descriptor-gen past the first x part's HWDGE so
        # the jt0 feed wins the first DMA-engine slot
        nudge = tpool.tile([P, 4], sb_dt, tag="nudge")
        for _ in range(5):
            nc.gpsimd.memset(nudge[:], 0.0)
        nc.gpsimd.dma_start(wk_sb[:], wk_d[:])

        cos_sb = tpool.tile([P, T], sb_dt, tag="cos")
        sin_sb = tpool.tile([P, T], sb_dt, tag="sin")
        tri_sb = tpool.tile([P, P], sb_dt, tag="tri")

        def load_tables():
            # emitted after the jt0 wq/x stream: these aren't needed until
            # the v-projection / RoPE / mask stages (~18us in), and early
            # emission steals DMA-engine slots from the jt0 feed
            nc.scalar.dma_start(wv_sb[:], wv_d[:])
            nc.scalar.dma_start(cos_sb[:], cos_d[:])
            nc.scalar.dma_start(sin_sb[:], sin_d[:])
            nc.gpsimd.dma_start(tri_sb[:], tri_d[:])
        # output-projection weights; loaded per-jc slice during B(0) so the
        # transfers don't compete with the jt0 x-chunk stream
        wo_sb = wpool.tile([P, NH, C], sb_dt, tag="wo")

        qrot = state.tile([P, NH, T], sb_dt, tag="qrot")
        krot = state.tile([P, T], sb_dt, tag="krot")
        v_sb = state.tile([P, T // P, D], sb_dt, tag="v")
        ot_sb = state.tile([P, NH, T], sb_dt, tag="ot")

        def load_x_part(xch, jt, k0, k1):
            nc.sync.dma_start(
                xch[:, k0:k1, :],
                xT_d[k0 * P:k1 * P,
                     jt * TCH:(jt + 1) * TCH].rearrange(
                    "(ko p) t -> p ko t", p=P))

        def load_x(jt):
            """Stream one [C, TCH] x chunk in a few multi-k-tile DMAs."""
            xch = xpool.tile([P, NK, TCH], sb_dt, tag="x", name=f"x{jt}")
            for k0, k1 in zip([0, 4, 8, 12], [4, 8, 12, 16]):
                load_x_part(xch, jt, k0, k1)
            return xch

        def rope(qall, qsw, o, out_ap, jt):
            # the half-swap was DMA'd once for the whole 5-output block
            # (engines cannot read two SBUF operands at different base
            # partitions); all DVE ops stay on the fast 2-byte path
            ch = slice(jt * TCH, (jt + 1) * TCH)
            m1 = ropep.tile([P, TCH], sb_dt, tag="m1")
            m2 = ropep.tile([P, TCH], sb_dt, tag="m2")
            nc.vector.tensor_tensor(
                m1[:], qall[:, o, :], cos_sb[:, ch], ALU.mult)
            nc.vector.tensor_tensor(
                m2[:], qsw[:, o, :], sin_sb[:, ch], ALU.mult)
            nc.vector.tensor_tensor(out_ap, m1[:], m2[:], ALU.add)

        def qk_finish(accs, jt):
            """Copy the 5 projection accumulators to fp16, swap-DMA the
            whole block once, then RoPE all 5 outputs."""
            ch = slice(jt * TCH, (jt + 1) * TCH)
            qall = qkp.tile([P, 5, TCH], sb_dt, tag="qk")
            for o in (4, 0, 1, 2, 3):
                nc.scalar.activation(qall[:, o, :], accs[o][:], AF.Copy)
            qsw = qkp.tile([P, 5, TCH], sb_dt, tag="qsw")
            nc.sync.dma_start(qsw[0:64, :, :], qall[64:128, :, :])
            nc.sync.dma_start(qsw[64:128, :, :], qall[0:64, :, :])
            for o in (4, 0, 1, 2, 3):
                rope(qall, qsw, o, krot[:, ch] if o == 4
                     else qrot[:, o, ch], jt)

        def w_slice(o, k):
            # output index o: 0..3 = q heads, 4 = k
            if o < NH:
                return wq_sb[:, k, o * D:(o + 1) * D]
            return wk_sb[:, k, :]

        ys_pending = {}

        def emit_C1(jt_c, tl, jc):
            """One output-projection tile: row-block tt = 4*jt_c + tl.
            y rows go out in jc pairs to halve the DMA instruction count."""
            tt = jt_c * (TCH // P) + tl
            yp = psA.tile([P, TCH], F32, tag="acc", name="yp")
            for h in range(NH):
                nc.tensor.matmul(
                    yp[:],
                    ot_sb[:, h, tt * P:(tt + 1) * P],
                    wo_sb[:, h, jc * TCH:(jc + 1) * TCH],
                    start=(h == 0), stop=(h == NH - 1))
            if tt == T // P - 1:  # final row-block: minimize drain
                ys = ysp.tile([P, 2, TCH], sb_dt, tag="ys")
                if jc % 2 == 0:
                    nc.vector.tensor_copy(ys[:, 0, :], yp[:])
                else:
                    nc.scalar.activation(ys[:, 0, :], yp[:], AF.Copy)
                nc.sync.dma_start(
                    y_d[tt * P:(tt + 1) * P, jc * TCH:(jc + 1) * TCH],
                    ys[:, 0, :])
            elif jc % 2 == 0:
                ys = ysp.tile([P, 2, TCH], sb_dt, tag="ys")
                ys_pending[tt] = ys
                nc.vector.tensor_copy(ys[:, 0, :], yp[:])
            else:
                ys = ys_pending.pop(tt)
                nc.scalar.activation(ys[:, 1, :], yp[:], AF.Copy)
                nc.sync.dma_start(
                    y_d[tt * P:(tt + 1) * P,
                        (jc - 1) * TCH:(jc + 1) * TCH],
                    ys[:, :, :])

        # jt0 feed, interleaved in k-consumption order
        xch_cur = xpool.tile([P, NK, TCH], sb_dt, tag="x", name="x0")
        load_wq(0, 1, nc.sync)
        load_x_part(xch_cur, 0, 0, 1)
        load_wq(1, 2, nc.scalar)
        load_x_part(xch_cur, 0, 1, 2)
        load_wq(2, 3, nc.scalar)
        load_x_part(xch_cur, 0, 2, 4)
        load_wq(3, 4, nc.scalar)
        load_wq(4, 6, nc.scalar)
        load_x_part(xch_cur, 0, 4, 8)
        load_wq(6, 8, nc.scalar)
        load_wq(8, 10, nc.scalar)
        load_x_part(xch_cur, 0, 8, 12)
        load_wq(10, 12, nc.scalar)
        load_wq(12, 14, nc.scalar)
        load_x_part(xch_cur, 0, 12, 16)
        load_wq(14, 16, nc.scalar)
        load_tables()

        for jt in range(NJT):
            ch = slice(jt * TCH, (jt + 1) * TCH)
            xs = xch_cur

            # ---- A(jt): q/k projections + RoPE, v in [t, d] layout ----
            if "A" in phases:
                if jt == 0:
                    # k-outer: consume wq chunks as they stream in.
                    # o>=2 accumulate in halves of the psS pair tiles.
                    sp0 = psS.tile([P, 2, TCH], F32, tag="s", name="accs0")
                    sp1 = psS.tile([P, 2, TCH], F32, tag="s", name="accs1")
                    accs = {0: psA.tile([P, TCH], F32, tag="acc", name="acc0"),
                            1: psA.tile([P, TCH], F32, tag="acc", name="acc1"),
                            2: sp0[:, 0, :], 3: sp0[:, 1, :], 4: sp1[:, 0, :]}
                    for k in range(NK):
                        for o in range(5):
                            nc.tensor.matmul(
                                accs[o][:], w_slice(o, k), xs[:, k, :],
                                start=(k == 0), stop=(k == NK - 1))
                    qk_finish(accs, jt)
                else:
                    # output-major: k and two q heads on the psA ring, the
                    # other two q heads in halves of one psS pair tile
                    sp = psS.tile([P, 2, TCH], F32, tag="s", name="accp")
                    accs = {}
                    for i, o in enumerate((4, 0, 1, 2, 3)):
                        if o in (2, 3):
                            accs[o] = sp[:, o - 2, :]
                        else:
                            accs[o] = psA.tile(
                                [P, TCH], F32, tag="acc", name=f"acc{o}")
                        for k in range(NK):
                            nc.tensor.matmul(
                                accs[o][:], w_slice(o, k), xs[:, k, :],
                                start=(k == 0), stop=(k == NK - 1))
                    qk_finish(accs, jt)

                if jt + 1 < NJT:
                    xch_cur = load_x(jt + 1)

                # v: x chunk stationary, wv moving
                for tl in range(TCH // P):
                    tt = jt * (TCH // P) + tl
                    vt_ps = psA.tile([P, D], F32, tag="acc", name="vt")
                    for k in range(NK):
                        nc.tensor.matmul(
                            vt_ps[:], xs[:, k, tl * P:(tl + 1) * P],
                            wv_sb[:, k, :],
                            start=(k == 0), stop=(k == NK - 1))
                    nc.scalar.activation(v_sb[:, tt, :], vt_ps[:], AF.Copy)

            # ---- B(jt): attention; C(jt-1) interleaved per head -------
            if "B" in phases:
                if jt == 0:
                    for jc in range(NJT):
                        nc.scalar.dma_start(
                            wo_sb[:, :, jc * TCH:(jc + 1) * TCH],
                            wo_d[:, jc * TCH:(jc + 1) * TCH].rearrange(
                                "(h p) c -> p h c", p=P))
                njs = 4 * jt + 4
                for h in range(NH):
                    ot_ps = psO.tile([P, TCH], F32, tag="ot")
                    qch = qrot[:, h, ch]
                    dn = dnp.tile([P, TCH], sb_dt, tag="dn")
                    cq = list(range(NJT)) if jt > 0 else []  # pending C jc's

                    def emit_pv(es, js, half, toff, njs=njs, ot_ps=ot_ps):
                        nc.tensor.matmul(
                            ot_ps[:, toff:], v_sb[:, js, :],
                            es[:, half, toff:],
                            start=(js == 0), stop=(js == njs - 1),
                            skip_group_check=True)

                    pend = []  # deferred so exp latency is hidden
                    for jp in range(njs // 2):
                        sp = psS.tile([P, 2, TCH], F32, tag="s")
                        halves = []
                        for half in (0, 1):
                            js = 2 * jp + half
                            toff = P * (js - 4 * jt) if js >= 4 * jt else 0
                            nc.tensor.matmul(
                                sp[:, half, toff:],
                                krot[:, js * P:(js + 1) * P], qch[:, toff:],
                                start=True, stop=True, skip_group_check=True)
                            halves.append((js, toff))
                        es = esp.tile([P, 2, TCH], sb_dt, tag="es")
                        if halves[0][1] == 0 and halves[1][1] == 0:
                            nc.scalar.activation(
                                es[:, :, :], sp[:, :, :], AF.Exp, scale=SCALE)
                        else:  # diagonal halves: exp only the computed part
                            for half, (js, toff) in enumerate(halves):
                                nc.scalar.activation(
                                    es[:, half, toff:], sp[:, half, toff:],
                                    AF.Exp, scale=SCALE)
                        for half, (js, toff) in enumerate(halves):
                            if js >= 4 * jt:  # diagonal: triangular mask
                                nc.vector.tensor_tensor(
                                    es[:, half, toff:toff + P],
                                    es[:, half, toff:toff + P],
                                    tri_sb[:], ALU.mult)
                            if js == 0:
                                nc.vector.tensor_copy(dn[:], es[:, half, :])
                            else:
                                nc.vector.tensor_tensor(
                                    dn[:, toff:], dn[:, toff:],
                                    es[:, half, toff:], ALU.add)
                            if len(pend) >= 4:
                                emit_pv(*pend.pop(0))
                            pend.append((es, js, half, toff))
                        # fill with prev-chunk out-proj; on the last chunk
                        # hold two tiles back for the post-flush bubble
                        if cq and (jt < 3 or jp in (2, 4)):
                            emit_C1(jt - 1, h, cq.pop(0))
                    for p_ in pend:
                        emit_pv(*p_)
                    for jc in cq:
                        emit_C1(jt - 1, h, jc)

                    den = dnp.tile([P, TCH], sb_dt, tag="den")
                    nc.gpsimd.partition_all_reduce(
                        den[:], dn[:], 128, bass_isa.ReduceOp.add)
                    rec = dnp.tile([P, TCH], sb_dt, tag="rec")
                    with nc.allow_low_precision(reason="fp16 softmax denom"):
                        nc.vector.reciprocal(rec[:], den[:])
                    nc.vector.tensor_tensor(
                        ot_sb[:, h, ch], ot_ps[:], rec[:], ALU.mult)


        # ---- C(3): final chunk's output projection --------------------
        if "C" in phases:
            for tl in range(TCH // P):
                for jc in range(NJT):
                    emit_C1(NJT - 1, tl, jc)

    nc.compile()
    return nc


def host_prep(x, wq, wk, wv, wo, mode="f16"):
    """Build the 8 per-core input maps (numpy, host-side reshuffles only)."""
    ndt = _np_dt(mode)
    x = np.asarray(x, dtype=np.float32)
    wq = np.asarray(wq, dtype=np.float32)
    wk = np.asarray(wk, dtype=np.float32)
    wv = np.asarray(wv, dtype=np.float32)
    wo = np.asarray(wo, dtype=np.float32)

    # RoPE even/odd grouping permutation within each head
    perm = np.concatenate([np.arange(0, D, 2), np.arange(1, D, 2)])

    # rope tables, transposed layout [d, t]
    inv_freq = (1.0 / THETA ** (np.arange(0, D, 2, dtype=np.float32) / D)
                ).astype(np.float32)
    pos = np.arange(T, dtype=np.float32)
    freqs = pos[:, None] * inv_freq[None, :]          # [T, 64] f32
    cos_t = np.cos(freqs).astype(np.float32).T        # [64, T]
    sin_t = np.sin(freqs).astype(np.float32).T        # [64, T]
    cosT = np.concatenate([cos_t, cos_t], axis=0)     # [128, T]
    sinT = np.concatenate([-sin_t, sin_t], axis=0)    # [128, T]

    # triangular multiplicative mask for diagonal blocks: allow f >= p
    f = np.arange(P)[None, :]
    p = np.arange(P)[:, None]
    tri = (f >= p).astype(np.float32)

    xTs = [np.ascontiguousarray(x[b].T).astype(ndt) for b in range(B)]

    in_maps = []
    for c in range(N_CORES):
        b, g = divmod(c, GROUP)
        rows = []
        for hh in range(NH):
            h = g * GROUP + hh
            rows.append(wq[h * D + perm, :])
        wq_g = np.concatenate(rows, axis=0)                  # [512, C]
        wk_g = wk[g * D + perm, :]                           # [128, C]
        wv_g = wv[g * D:(g + 1) * D, :]                      # [128, C]
        wo_g = wo[:, g * NH * D:(g + 1) * NH * D]            # [C, 512]

        in_maps.append({
            "xT": xTs[b],
            "wqT": np.ascontiguousarray(wq_g.T).astype(ndt),
            "wkT": np.ascontiguousarray(
                wk_g.T.reshape(NK, P, D).transpose(1, 0, 2)).astype(ndt),
            "wvT": np.ascontiguousarray(
                wv_g.T.reshape(NK, P, D).transpose(1, 0, 2)).astype(ndt),
            "woT": np.ascontiguousarray(wo_g.T).astype(ndt),
            "cosT": cosT.astype(ndt),
            "sinT": sinT.astype(ndt),
            "triT": tri.astype(ndt),
        })
    return in_maps


_CACHE = {}


def _get_program(mode):
    if mode not in _CACHE:
        _CACHE[mode] = build_program(mode)
    return _CACHE[mode]


def kernel(x, mask, wq, wk, wv, wo):
    mode = os.environ.get("BASS_ATTN_MODE", "f16")
    nc = _get_program(mode)
    in_maps = host_prep(x, wq, wk, wv, wo, mode)
    res = run_bass_kernel_spmd(nc, in_maps, list(range(N_CORES))).results
    out = np.zeros((B, T, C), dtype=np.float32)
    for c in range(N_CORES):
        out[c // GROUP] += res[c]["y"]
    return out


# revision 22
# speedup vs baseline: 1.0012x; 1.0012x over previous
"""Causal GQA attention block (B=2, T=2048, C=2048, H=16, HKV=4, D=128, RoPE)
on 8 Trainium2 NeuronCores.

Sharding: core c handles batch b = c//4 and kv-group g = c%4 (4 q heads +
1 kv head per core).  The output projection is row-parallel: each core
produces a partial [T, C] contribution; the host sums the 4 partials per
batch.

v2 design (single fused jt-pipelined loop, fp16 storage):
  - all SBUF tensors fp16 (magnitudes here stay < 1e4, fp16 keeps per-elem
    rel err ~5e-4; matmuls run at 1 cycle/row like bf16).
  - x is host-transposed to xT [C, T]; q/k projections produce [D, T]
    head-transposed tiles; RoPE applied via host-side even/odd permutation
    folded into wq/wk + partition-swapped multiplies.
  - v is produced directly in [T, D] layout (x chunk as the stationary
    operand, wv as moving) -- no PE transposes at all.
  - scores are computed transposed (S.T tiles [s, t]); causal structure is
    exploited at fine grain: diagonal s-tiles only compute the t >= s part
    (moving dim shortened to 512-128r), with a single [128,128] triangular
    multiplicative mask for the crossing block.
  - softmax: no max-subtraction (scores are O(5), exp safe in fp32 PSUM);
    1/sqrt(D) folded into the exp's scale argument (free on ACT).
    The denominator is accumulated on DVE (fp16 adds of exp'd tiles) and
    partition-reduced+broadcast in one gpsimd partition_all_reduce -- no
    tensor-engine work.
  - the output projection for chunk jt-1 is interleaved into the attention
    h-loop of chunk jt (one tt row-block per head) so its matmuls fill the
    attention phase's dependency bubbles; it reuses the scores PSUM ring.
  - x chunks stream in 4-contraction-tile DMAs (few, large transfers --
    the cost model serializes descriptor generation per DMA instruction).
"""

import os
from contextlib import ExitStack

import numpy as np

import concourse.bass as bass
import concourse.tile as tile
from concourse import bass_isa
from concourse import bacc, mybir
from concourse.bass_utils import run_bass_kernel_spmd

# problem constants
B, T, C = 2, 2048, 2048
H, HKV, D = 16, 4, 128
GROUP = H // HKV           # 4 q heads per kv head
THETA = 1000000.0
SCALE = D ** -0.5

P = 128                    # partitions
TCH = 512                  # t-chunk (matmul moving free dim)
NJT = T // TCH             # 4 t-chunks
NK = C // P                # 16 contraction tiles
NH = GROUP                 # 4 local q heads
N_CORES = 8

F32 = mybir.dt.float32
AF = mybir.ActivationFunctionType
ALU = mybir.AluOpType


def _sb_dt(mode):
    return {"f16": mybir.dt.float16, "bf16": mybir.dt.bfloat16}[mode]


def _np_dt(mode):
    if mode == "bf16":
        import ml_dtypes
        return ml_dtypes.bfloat16
    return np.float16


def build_program(mode="f16", phases="ABC", variant=""):
    """Build and compile the per-core Bass program. Returns nc."""
    sb_dt = _sb_dt(mode)

    nc = bacc.Bacc("TRN2", target_bir_lowering=False, debug=False)

    xT_d = nc.dram_tensor("xT", [C, T], sb_dt, kind="ExternalInput").ap()
    wq_d = nc.dram_tensor("wqT", [C, NH * D], sb_dt, kind="ExternalInput").ap()
    wk_d = nc.dram_tensor("wkT", [P, NK, D], sb_dt, kind="ExternalInput").ap()
    wv_d = nc.dram_tensor("wvT", [P, NK, D], sb_dt, kind="ExternalInput").ap()
    wo_d = nc.dram_tensor("woT", [NH * D, C], sb_dt, kind="ExternalInput").ap()
    cos_d = nc.dram_tensor("cosT", [P, T], sb_dt, kind="ExternalInput").ap()
    sin_d = nc.dram_tensor("sinT", [P, T], sb_dt, kind="ExternalInput").ap()
    tri_d = nc.dram_tensor("triT", [P, P], sb_dt, kind="ExternalInput").ap()
    y_d = nc.dram_tensor("y", [T, C], sb_dt, kind="ExternalOutput").ap()

    with tile.TileContext(nc) as tc, ExitStack() as ctx:
        wpool = ctx.enter_context(tc.tile_pool(name="weights", bufs=1))
        tpool = ctx.enter_context(tc.tile_pool(name="tables", bufs=1))
        state = ctx.enter_context(tc.tile_pool(name="state", bufs=1))
        xpool = ctx.enter_context(tc.tile_pool(name="xsub", bufs=2))
        qkp = ctx.enter_context(tc.tile_pool(name="qkstage", bufs=3))
        ropep = ctx.enter_context(tc.tile_pool(name="rope", bufs=2))
        esp = ctx.enter_context(tc.tile_pool(name="es", bufs=8))
        dnp = ctx.enter_context(tc.tile_pool(name="dn", bufs=2))
        ysp = ctx.enter_context(tc.tile_pool(name="ys", bufs=4))
        psA = ctx.enter_context(tc.tile_pool(name="psA", bufs=2, space="PSUM"))
        psS = ctx.enter_context(tc.tile_pool(name="psS", bufs=2, space="PSUM"))
        psO = ctx.enter_context(tc.tile_pool(name="psO", bufs=2, space="PSUM"))

        # ---- weight / table loads -------------------------------------
        wq_sb = wpool.tile([P, NK, NH * D], sb_dt, tag="wq")
        wk_sb = wpool.tile([P, NK, D], sb_dt, tag="wk")
        wv_sb = wpool.tile([P, NK, D], sb_dt, tag="wv")
        def load_wq(k0, k1, eng):
            eng.dma_start(
                wq_sb[:, k0:k1, :],
                wq_d[k0 * P:k1 * P, :].rearrange("(ko p) o -> p ko o", p=P))
        # nudge wk's SWDGE descriptor-gen past the first x part's HWDGE so
        # the jt0 feed wins the first DMA-engine slot
        nudge = tpool.tile([P, 4], sb_dt, tag="nudge")
        for _ in range(5):
            nc.gpsimd.memset(nudge[:], 0.0)
        nc.gpsimd.dma_start(wk_sb[:], wk_d[:])

        cos_sb = tpool.tile([P, T], sb_dt, tag="cos")
        sin_sb = tpool.tile([P, T], sb_dt, tag="sin")
        tri_sb = tpool.tile([P, P], sb_dt, tag="tri")

        def load_tables():
            # emitted after the jt0 wq/x stream: these aren't needed until
            # the v-projection / RoPE / mask stages (~18us in), and early
            # emission steals DMA-engine slots from the jt0 feed
            nc.scalar.dma_start(wv_sb[:], wv_d[:])
            nc.scalar.dma_start(cos_sb[:], cos_d[:])
            nc.scalar.dma_start(sin_sb[:], sin_d[:])
            nc.gpsimd.dma_start(tri_sb[:], tri_d[:])
        # output-projection weights; loaded per-jc slice during B(0) so the
        # transfers don't compete with the jt0 x-chunk stream
        wo_sb = wpool.tile([P, NH, C], sb_dt, tag="wo")

        qrot = state.tile([P, NH, T], sb_dt, tag="qrot")
        krot = state.tile([P, T], sb_dt, tag="krot")
        v_sb = state.tile([P, T // P, D], sb_dt, tag="v")
        ot_sb = state.tile([P, NH, T], sb_dt, tag="ot")

        def load_x_part(xch, jt, k0, k1):
            nc.sync.dma_start(
                xch[:, k0:k1, :],
                xT_d[k0 * P:k1 * P,
                     jt * TCH:(jt + 1) * TCH].rearrange(
                    "(ko p) t -> p ko t", p=P))

        def load_x(jt):
            """Stream one [C, TCH] x chunk in a few multi-k-tile DMAs."""
            xch = xpool.tile([P, NK, TCH], sb_dt, tag="x", name=f"x{jt}")
            for k0, k1 in zip([0, 4, 8, 12], [4, 8, 12, 16]):
                load_x_part(xch, jt, k0, k1)
            return xch

        def rope(qall, qsw, o, out_ap, jt):
            # the half-swap was DMA'd once for the whole 5-output block
            # (engines cannot read two SBUF operands at different base
            # partitions); all DVE ops stay on the fast 2-byte path
            ch = slice(jt * TCH, (jt + 1) * TCH)
            m1 = ropep.tile([P, TCH], sb_dt, tag="m1")
            m2 = ropep.tile([P, TCH], sb_dt, tag="m2")
            nc.vector.tensor_tensor(
                m1[:], qall[:, o, :], cos_sb[:, ch], ALU.mult)
            nc.vector.tensor_tensor(
                m2[:], qsw[:, o, :], sin_sb[:, ch], ALU.mult)
            nc.vector.tensor_tensor(out_ap, m1[:], m2[:], ALU.add)

        def qk_finish(accs, jt):
            """Copy the 5 projection accumulators to fp16, swap-DMA the
            whole block once, then RoPE all 5 outputs."""
            ch = slice(jt * TCH, (jt + 1) * TCH)
            qall = qkp.tile([P, 5, TCH], sb_dt, tag="qk")
            for o in (4, 0, 1, 2, 3):
                nc.scalar.activation(qall[:, o, :], accs[o][:], AF.Copy)
            qsw = qkp.tile([P, 5, TCH], sb_dt, tag="qsw")
            nc.sync.dma_start(qsw[0:64, :, :], qall[64:128, :, :])
            nc.sync.dma_start(qsw[64:128, :, :], qall[0:64, :, :])
            for o in (4, 0, 1, 2, 3):
                rope(qall, qsw, o, krot[:, ch] if o == 4
                     else qrot[:, o, ch], jt)

        def w_slice(o, k):
            # output index o: 0..3 = q heads, 4 = k
            if o < NH:
                return wq_sb[:, k, o * D:(o + 1) * D]
            return wk_sb[:, k, :]

        ys_pending = {}

        def emit_C1(jt_c, tl, jc):
            """One output-projection tile: row-block tt = 4*jt_c + tl.
            y rows go out in jc pairs to halve the DMA instruction count."""
            tt = jt_c * (TCH // P) + tl
            yp = psA.tile([P, TCH], F32, tag="acc", name="yp")
            for h in range(NH):
                nc.tensor.matmul(
                    yp[:],
                    ot_sb[:, h, tt * P:(tt + 1) * P],
                    wo_sb[:, h, jc * TCH:(jc + 1) * TCH],
                    start=(h == 0), stop=(h == NH - 1))
            if tt == T // P - 1:  # final row-block: minimize drain
                ys = ysp.tile([P, 2, TCH], sb_dt, tag="ys")
                if jc % 2 == 0:
                    nc.vector.tensor_copy(ys[:, 0, :], yp[:])
                else:
                    nc.scalar.activation(ys[:, 0, :], yp[:], AF.Copy)
                nc.sync.dma_start(
                    y_d[tt * P:(tt + 1) * P, jc * TCH:(jc + 1) * TCH],
                    ys[:, 0, :])
            elif jc % 2 == 0:
                ys = ysp.tile([P, 2, TCH], sb_dt, tag="ys")
                ys_pending[tt] = ys
                nc.vector.tensor_copy(ys[:, 0, :], yp[:])
            else:
                ys = ys_pending.pop(tt)
                nc.scalar.activation(ys[:, 1, :], yp[:], AF.Copy)
                nc.sync.dma_start(
                    y_d[tt * P:(tt + 1) * P,
                        (jc - 1) * TCH:(jc + 1) * TCH],
                    ys[:, :, :])

        # jt0 feed, interleaved in k-consumption order
        xch_cur = xpool.tile([P, NK, TCH], sb_dt, tag="x", name="x0")
        load_wq(0, 1, nc.sync)
        load_x_part(xch_cur, 0, 0, 1)
        load_wq(1, 2, nc.scalar)
        load_x_part(xch_cur, 0, 1, 2)
        load_wq(2, 3, nc.scalar)
        load_x_part(xch_cur, 0, 2, 4)
        load_wq(3, 4, nc.scalar)
        load_wq(4, 6, nc.scalar)
        load_x_part(xch_cur, 0, 4, 8)
        load_wq(6, 8, nc.scalar)
        load_wq(8, 10, nc.scalar)
        load_x_part(xch_cur, 0, 8, 12)
        load_wq(10, 12, nc.scalar)
        load_wq(12, 14, nc.scalar)
        load_x_part(xch_cur, 0, 12, 16)
        load_wq(14, 16, nc.scalar)
        load_tables()

        for jt in range(NJT):
            ch = slice(jt * TCH, (jt + 1) * TCH)
            xs = xch_cur

            # ---- A(jt): q/k projections + RoPE, v in [t, d] layout ----
            if "A" in phases:
                if jt == 0:
                    # k-outer: consume wq chunks as they stream in.
                    # o>=2 accumulate in halves of the psS pair tiles.
                    sp0 = psS.tile([P, 2, TCH], F32, tag="s", name="accs0")
                    sp1 = psS.tile([P, 2, TCH], F32, tag="s", name="accs1")
                    accs = {0: psA.tile([P, TCH], F32, tag="acc", name="acc0"),
                            1: psA.tile([P, TCH], F32, tag="acc", name="acc1"),
                            2: sp0[:, 0, :], 3: sp0[:, 1, :], 4: sp1[:, 0, :]}
                    for k in range(NK):
                        for o in range(5):
                            nc.tensor.matmul(
                                accs[o][:], w_slice(o, k), xs[:, k, :],
                                start=(k == 0), stop=(k == NK - 1))
                    qk_finish(accs, jt)
                else:
                    # output-major: k and two q heads on the psA ring, the
                    # other two q heads in halves of one psS pair tile
                    sp = psS.tile([P, 2, TCH], F32, tag="s", name="accp")
                    accs = {}
                    for i, o in enumerate((4, 0, 1, 2, 3)):
                        if o in (2, 3):
                            accs[o] = sp[:, o - 2, :]
                        else:
                            accs[o] = psA.tile(
                                [P, TCH], F32, tag="acc", name=f"acc{o}")
                        for k in range(NK):
                            nc.tensor.matmul(
                                accs[o][:], w_slice(o, k), xs[:, k, :],
                                start=(k == 0), stop=(k == NK - 1))
                    qk_finish(accs, jt)

                if jt + 1 < NJT:
                    xch_cur = load_x(jt + 1)

                # v: x chunk stationary, wv moving
                for tl in range(TCH // P):
                    tt = jt * (TCH // P) + tl
                    vt_ps = psA.tile([P, D], F32, tag="acc", name="vt")
                    for k in range(NK):
                        nc.tensor.matmul(
                            vt_ps[:], xs[:, k, tl * P:(tl + 1) * P],
                            wv_sb[:, k, :],
                            start=(k == 0), stop=(k == NK - 1))
                    nc.scalar.activation(v_sb[:, tt, :], vt_ps[:], AF.Copy)

            # ---- B(jt): attention; C(jt-1) interleaved per head -------
            if "B" in phases:
                if jt == 0:
                    for jc in range(NJT):
                        nc.scalar.dma_start(
                            wo_sb[:, :, jc * TCH:(jc + 1) * TCH],
                            wo_d[:, jc * TCH:(jc + 1) * TCH].rearrange(
                                "(h p) c -> p h c", p=P))
                njs = 4 * jt + 4
                for h in range(NH):
                    ot_ps = psO.tile([P, TCH], F32, tag="ot")
                    qch = qrot[:, h, ch]
                    dn = dnp.tile([P, TCH], sb_dt, tag="dn")
                    cq = list(range(NJT)) if jt > 0 else []  # pending C jc's

                    def emit_pv(es, js, half, toff, njs=njs, ot_ps=ot_ps):
                        nc.tensor.matmul(
                            ot_ps[:, toff:], v_sb[:, js, :],
                            es[:, half, toff:],
                            start=(js == 0), stop=(js == njs - 1),
                            skip_group_check=True)

                    pend = []  # deferred so exp latency is hidden
                    for jp in range(njs // 2):
                        sp = psS.tile([P, 2, TCH], F32, tag="s")
                        halves = []
                        for half in (0, 1):
                            js = 2 * jp + half
                            toff = P * (js - 4 * jt) if js >= 4 * jt else 0
                            nc.tensor.matmul(
                                sp[:, half, toff:],
                                krot[:, js * P:(js + 1) * P], qch[:, toff:],
                                start=True, stop=True, skip_group_check=True)
                            halves.append((js, toff))
                        es = esp.tile([P, 2, TCH], sb_dt, tag="es")
                        if halves[0][1] == 0 and halves[1][1] == 0:
                            nc.scalar.activation(
                                es[:, :, :], sp[:, :, :], AF.Exp, scale=SCALE)
                        else:  # diagonal halves: exp only the computed part
                            for half, (js, toff) in enumerate(halves):
                                nc.scalar.activation(
                                    es[:, half, toff:], sp[:, half, toff:],
                                    AF.Exp, scale=SCALE)
                        for half, (js, toff) in enumerate(halves):
                            if js >= 4 * jt:  # diagonal: triangular mask
                                nc.vector.tensor_tensor(
                                    es[:, half, toff:toff + P],
                                    es[:, half, toff:toff + P],
                                    tri_sb[:], ALU.mult)
                            if js == 0:
                                nc.vector.tensor_copy(dn[:], es[:, half, :])
                            else:
                                nc.vector.tensor_tensor(
                                    dn[:, toff:], dn[:, toff:],
                                    es[:, half, toff:], ALU.add)
                            if len(pend) >= 4:
                                emit_pv(*pend.pop(0))
                            pend.append((es, js, half, toff))
                        if cq:  # fill with prev-chunk out-proj
                            emit_C1(jt - 1, h, cq.pop(0))
                    for p_ in pend:
                        emit_pv(*p_)
                    for jc in cq:
                        emit_C1(jt - 1, h, jc)

                    den = dnp.tile([P, TCH], sb_dt, tag="den")
                    nc.gpsimd.partition_all_reduce(
                        den[:], dn[:], 128, bass_isa.ReduceOp.add)
                    rec = dnp.tile([P, TCH], sb_dt, tag="rec")
                    with nc.allow_low_precision(reason="fp16 softmax denom"):
                        nc.vector.reciprocal(rec[:], den[:])
                    nc.vector.tensor_tensor(
                        ot_sb[:, h, ch], ot_ps[:], rec[:], ALU.mult)


        # ---- C(3): final chunk's output projection --------------------
        if "C" in phases:
            for tl in range(TCH // P):
                for jc in range(NJT):
                    emit_C1(NJT - 1, tl, jc)

    nc.compile()
    return nc


def host_prep(x, wq, wk, wv, wo, mode="f16"):
    """Build the 8 per-core input maps (numpy, host-side reshuffles only)."""
    ndt = _np_dt(mode)
    x = np.asarray(x, dtype=np.float32)
    wq = np.asarray(wq, dtype=np.float32)
    wk = np.asarray(wk, dtype=np.float32)
    wv = np.asarray(wv, dtype=np.float32)
    wo = np.asarray(wo, dtype=np.float32)

    # RoPE even/odd grouping permutation within each head
    perm = np.concatenate([np.arange(0, D, 2), np.arange(1, D, 2)])

    # rope tables, transposed layout [d, t]
    inv_freq = (1.0 / THETA ** (np.arange(0, D, 2, dtype=np.float32) / D)
                ).astype(np.float32)
    pos = np.arange(T, dtype=np.float32)
    freqs = pos[:, None] * inv_freq[None, :]          # [T, 64] f32
    cos_t = np.cos(freqs).astype(np.float32).T        # [64, T]
    sin_t = np.sin(freqs).astype(np.float32).T        # [64, T]
    cosT = np.concatenate([cos_t, cos_t], axis=0)     # [128, T]
    sinT = np.concatenate([-sin_t, sin_t], axis=0)    # [128, T]

    # triangular multiplicative mask for diagonal blocks: allow f >= p
    f = np.arange(P)[None, :]
    p = np.arange(P)[:, None]
    tri = (f >= p).astype(np.float32)

    xTs = [np.ascontiguousarray(x[b].T).astype(ndt) for b in range(B)]

    in_maps = []
    for c in range(N_CORES):
        b, g = divmod(c, GROUP)
        rows = []
        for hh in range(NH):
            h = g * GROUP + hh
            rows.append(wq[h * D + perm, :])
        wq_g = np.concatenate(rows, axis=0)                  # [512, C]
        wk_g = wk[g * D + perm, :]                           # [128, C]
        wv_g = wv[g * D:(g + 1) * D, :]                      # [128, C]
        wo_g = wo[:, g * NH * D:(g + 1) * NH * D]            # [C, 512]

        in_maps.append({
            "xT": xTs[b],
            "wqT": np.ascontiguousarray(wq_g.T).astype(ndt),
            "wkT": np.ascontiguousarray(
                wk_g.T.reshape(NK, P, D).transpose(1, 0, 2)).astype(ndt),
            "wvT": np.ascontiguousarray(
                wv_g.T.reshape(NK, P, D).transpose(1, 0, 2)).astype(ndt),
            "woT": np.ascontiguousarray(wo_g.T).astype(ndt),
            "cosT": cosT.astype(ndt),
            "sinT": sinT.astype(ndt),
            "triT": tri.astype(ndt),
        })
    return in_maps


_CACHE = {}


def _get_program(mode):
    if mode not in _CACHE:
        _CACHE[mode] = build_program(mode)
    return _CACHE[mode]


def kernel(x, mask, wq, wk, wv, wo):
    mode = os.environ.get("BASS_ATTN_MODE", "f16")
    nc = _get_program(mode)
    in_maps = host_prep(x, wq, wk, wv, wo, mode)
    res = run_bass_kernel_spmd(nc, in_maps, list(range(N_CORES))).results
    out = np.zeros((B, T, C), dtype=np.float32)
    for c in range(N_CORES):
        out[c // GROUP] += res[c]["y"]
    return out


# revision 24
# speedup vs baseline: 1.0072x; 1.0060x over previous
"""Causal GQA attention block (B=2, T=2048, C=2048, H=16, HKV=4, D=128, RoPE)
on 8 Trainium2 NeuronCores.

Sharding: core c handles batch b = c//4 and kv-group g = c%4 (4 q heads +
1 kv head per core).  The output projection is row-parallel: each core
produces a partial [T, C] contribution; the host sums the 4 partials per
batch.

v2 design (single fused jt-pipelined loop, fp16 storage):
  - all SBUF tensors fp16 (magnitudes here stay < 1e4, fp16 keeps per-elem
    rel err ~5e-4; matmuls run at 1 cycle/row like bf16).
  - x is host-transposed to xT [C, T]; q/k projections produce [D, T]
    head-transposed tiles; RoPE applied via host-side even/odd permutation
    folded into wq/wk + partition-swapped multiplies.
  - v is produced directly in [T, D] layout (x chunk as the stationary
    operand, wv as moving) -- no PE transposes at all.
  - scores are computed transposed (S.T tiles [s, t]); causal structure is
    exploited at fine grain: diagonal s-tiles only compute the t >= s part
    (moving dim shortened to 512-128r), with a single [128,128] triangular
    multiplicative mask for the crossing block.
  - softmax: no max-subtraction (scores are O(5), exp safe in fp32 PSUM);
    1/sqrt(D) folded into the exp's scale argument (free on ACT).
    The denominator is accumulated on DVE (fp16 adds of exp'd tiles) and
    partition-reduced+broadcast in one gpsimd partition_all_reduce -- no
    tensor-engine work.
  - the output projection for chunk jt-1 is interleaved into the attention
    h-loop of chunk jt (one tt row-block per head) so its matmuls fill the
    attention phase's dependency bubbles; it reuses the scores PSUM ring.
  - x chunks stream in 4-contraction-tile DMAs (few, large transfers --
    the cost model serializes descriptor generation per DMA instruction).
"""

import os
from contextlib import ExitStack

import numpy as np

import concourse.bass as bass
import concourse.tile as tile
from concourse import bass_isa
from concourse import bacc, mybir
from concourse.bass_utils import run_bass_kernel_spmd

# problem constants
B, T, C = 2, 2048, 2048
H, HKV, D = 16, 4, 128
GROUP = H // HKV           # 4 q heads per kv head
THETA = 1000000.0
SCALE = D ** -0.5

P = 128                    # partitions
TCH = 512                  # t-chunk (matmul moving free dim)
NJT = T // TCH             # 4 t-chunks
NK = C // P                # 16 contraction tiles
NH = GROUP                 # 4 local q heads
N_CORES = 8

F32 = mybir.dt.float32
AF = mybir.ActivationFunctionType
ALU = mybir.AluOpType


def _sb_dt(mode):
    return {"f16": mybir.dt.float16, "bf16": mybir.dt.bfloat16}[mode]


def _np_dt(mode):
    if mode == "bf16":
        import ml_dtypes
        return ml_dtypes.bfloat16
    return np.float16


def build_program(mode="f16", phases="ABC", variant=""):
    """Build and compile the per-core Bass program. Returns nc."""
    sb_dt = _sb_dt(mode)

    nc = bacc.Bacc("TRN2", target_bir_lowering=False, debug=False)

    xT_d = nc.dram_tensor("xT", [C, T], sb_dt, kind="ExternalInput").ap()
    wq_d = nc.dram_tensor("wqT", [C, NH * D], sb_dt, kind="ExternalInput").ap()
    wk_d = nc.dram_tensor("wkT", [P, NK, D], sb_dt, kind="ExternalInput").ap()
    wv_d = nc.dram_tensor("wvT", [P, NK, D], sb_dt, kind="ExternalInput").ap()
    wo_d = nc.dram_tensor("woT", [NH * D, C], sb_dt, kind="ExternalInput").ap()
    cos_d = nc.dram_tensor("cosT", [P, T], sb_dt, kind="ExternalInput").ap()
    sin_d = nc.dram_tensor("sinT", [P, T], sb_dt, kind="ExternalInput").ap()
    tri_d = nc.dram_tensor("triT", [P, P], sb_dt, kind="ExternalInput").ap()
    y_d = nc.dram_tensor("y", [T, C], sb_dt, kind="ExternalOutput").ap()

    with tile.TileContext(nc) as tc, ExitStack() as ctx:
        wpool = ctx.enter_context(tc.tile_pool(name="weights", bufs=1))
        tpool = ctx.enter_context(tc.tile_pool(name="tables", bufs=1))
        state = ctx.enter_context(tc.tile_pool(name="state", bufs=1))
        xpool = ctx.enter_context(tc.tile_pool(name="xsub", bufs=2))
        qkp = ctx.enter_context(tc.tile_pool(name="qkstage", bufs=3))
        ropep = ctx.enter_context(tc.tile_pool(name="rope", bufs=2))
        esp = ctx.enter_context(tc.tile_pool(name="es", bufs=8))
        dnp = ctx.enter_context(tc.tile_pool(name="dn", bufs=2))
        ysp = ctx.enter_context(tc.tile_pool(name="ys", bufs=4))
        psA = ctx.enter_context(tc.tile_pool(name="psA", bufs=2, space="PSUM"))
        psS = ctx.enter_context(tc.tile_pool(name="psS", bufs=2, space="PSUM"))
        psO = ctx.enter_context(tc.tile_pool(name="psO", bufs=2, space="PSUM"))

        # ---- weight / table loads -------------------------------------
        wq_sb = wpool.tile([P, NK, NH * D], sb_dt, tag="wq")
        wk_sb = wpool.tile([P, NK, D], sb_dt, tag="wk")
        wv_sb = wpool.tile([P, NK, D], sb_dt, tag="wv")
        def load_wq(k0, k1, eng):
            eng.dma_start(
                wq_sb[:, k0:k1, :],
                wq_d[k0 * P:k1 * P, :].rearrange("(ko p) o -> p ko o", p=P))
        nc.gpsimd.dma_start(wk_sb[:], wk_d[:])

        cos_sb = tpool.tile([P, T], sb_dt, tag="cos")
        sin_sb = tpool.tile([P, T], sb_dt, tag="sin")
        tri_sb = tpool.tile([P, P], sb_dt, tag="tri")

        def load_tables():
            # emitted after the jt0 wq/x stream: these aren't needed until
            # the v-projection / RoPE / mask stages (~18us in), and early
            # emission steals DMA-engine slots from the jt0 feed
            nc.scalar.dma_start(wv_sb[:], wv_d[:])
            nc.scalar.dma_start(cos_sb[:], cos_d[:])
            nc.scalar.dma_start(sin_sb[:], sin_d[:])
            nc.gpsimd.dma_start(tri_sb[:], tri_d[:])
        # output-projection weights; loaded per-jc slice during B(0) so the
        # transfers don't compete with the jt0 x-chunk stream
        wo_sb = wpool.tile([P, NH, C], sb_dt, tag="wo")

        qrot = state.tile([P, NH, T], sb_dt, tag="qrot")
        krot = state.tile([P, T], sb_dt, tag="krot")
        v_sb = state.tile([P, T // P, D], sb_dt, tag="v")
        ot_sb = state.tile([P, NH, T], sb_dt, tag="ot")

        def load_x_part(xch, jt, k0, k1):
            nc.sync.dma_start(
                xch[:, k0:k1, :],
                xT_d[k0 * P:k1 * P,
                     jt * TCH:(jt + 1) * TCH].rearrange(
                    "(ko p) t -> p ko t", p=P))

        def load_x(jt):
            """Stream one [C, TCH] x chunk in a few multi-k-tile DMAs."""
            xch = xpool.tile([P, NK, TCH], sb_dt, tag="x", name=f"x{jt}")
            for k0, k1 in zip([0, 4, 8, 12], [4, 8, 12, 16]):
                load_x_part(xch, jt, k0, k1)
            return xch

        def rope(qall, qsw, o, out_ap, jt):
            # the half-swap was DMA'd once for the whole 5-output block
            # (engines cannot read two SBUF operands at different base
            # partitions); all DVE ops stay on the fast 2-byte path
            ch = slice(jt * TCH, (jt + 1) * TCH)
            m1 = ropep.tile([P, TCH], sb_dt, tag="m1")
            m2 = ropep.tile([P, TCH], sb_dt, tag="m2")
            nc.vector.tensor_tensor(
                m1[:], qall[:, o, :], cos_sb[:, ch], ALU.mult)
            nc.vector.tensor_tensor(
                m2[:], qsw[:, o, :], sin_sb[:, ch], ALU.mult)
            nc.vector.tensor_tensor(out_ap, m1[:], m2[:], ALU.add)

        def qk_finish(accs, jt):
            """Copy the 5 projection accumulators to fp16, swap-DMA the
            whole block once, then RoPE all 5 outputs."""
            ch = slice(jt * TCH, (jt + 1) * TCH)
            qall = qkp.tile([P, 5, TCH], sb_dt, tag="qk")
            for o in (4, 0, 1, 2, 3):
                nc.scalar.activation(qall[:, o, :], accs[o][:], AF.Copy)
            qsw = qkp.tile([P, 5, TCH], sb_dt, tag="qsw")
            nc.sync.dma_start(qsw[0:64, :, :], qall[64:128, :, :])
            nc.sync.dma_start(qsw[64:128, :, :], qall[0:64, :, :])
            for o in (4, 0, 1, 2, 3):
                rope(qall, qsw, o, krot[:, ch] if o == 4
                     else qrot[:, o, ch], jt)

        def w_slice(o, k):
            # output index o: 0..3 = q heads, 4 = k
            if o < NH:
                return wq_sb[:, k, o * D:(o + 1) * D]
            return wk_sb[:, k, :]

        ys_pending = {}

        def emit_C1(jt_c, tl, jc):
            """One output-projection tile: row-block tt = 4*jt_c + tl.
            y rows go out in jc pairs to halve the DMA instruction count."""
            tt = jt_c * (TCH // P) + tl
            yp = psA.tile([P, TCH], F32, tag="acc", name="yp")
            for h in range(NH):
                nc.tensor.matmul(
                    yp[:],
                    ot_sb[:, h, tt * P:(tt + 1) * P],
                    wo_sb[:, h, jc * TCH:(jc + 1) * TCH],
                    start=(h == 0), stop=(h == NH - 1))
            if jc % 2 == 0:
                ys = ysp.tile([P, 2, TCH], sb_dt, tag="ys")
                ys_pending[tt] = ys
                nc.vector.tensor_copy(ys[:, 0, :], yp[:])
            else:
                ys = ys_pending.pop(tt)
                nc.scalar.activation(ys[:, 1, :], yp[:], AF.Copy)
                nc.sync.dma_start(
                    y_d[tt * P:(tt + 1) * P,
                        (jc - 1) * TCH:(jc + 1) * TCH],
                    ys[:, :, :])

        # jt0 feed, interleaved in k-consumption order
        xch_cur = xpool.tile([P, NK, TCH], sb_dt, tag="x", name="x0")
        load_wq(0, 1, nc.sync)
        load_x_part(xch_cur, 0, 0, 1)
        load_wq(1, 2, nc.scalar)
        load_x_part(xch_cur, 0, 1, 2)
        load_wq(2, 3, nc.scalar)
        load_x_part(xch_cur, 0, 2, 4)
        load_wq(3, 4, nc.scalar)
        load_wq(4, 6, nc.scalar)
        load_x_part(xch_cur, 0, 4, 8)
        load_wq(6, 8, nc.scalar)
        load_wq(8, 10, nc.scalar)
        load_x_part(xch_cur, 0, 8, 12)
        load_wq(10, 12, nc.scalar)
        load_wq(12, 14, nc.scalar)
        load_x_part(xch_cur, 0, 12, 16)
        load_wq(14, 16, nc.scalar)
        load_tables()

        for jt in range(NJT):
            ch = slice(jt * TCH, (jt + 1) * TCH)
            xs = xch_cur

            # ---- A(jt): q/k projections + RoPE, v in [t, d] layout ----
            if "A" in phases:
                if jt == 0:
                    # k-outer: consume wq chunks as they stream in.
                    # o>=2 accumulate in halves of the psS pair tiles.
                    sp0 = psS.tile([P, 2, TCH], F32, tag="s", name="accs0")
                    sp1 = psS.tile([P, 2, TCH], F32, tag="s", name="accs1")
                    accs = {0: psA.tile([P, TCH], F32, tag="acc", name="acc0"),
                            1: psA.tile([P, TCH], F32, tag="acc", name="acc1"),
                            2: sp0[:, 0, :], 3: sp0[:, 1, :], 4: sp1[:, 0, :]}
                    for k in range(NK):
                        for o in range(5):
                            nc.tensor.matmul(
                                accs[o][:], w_slice(o, k), xs[:, k, :],
                                start=(k == 0), stop=(k == NK - 1))
                    qk_finish(accs, jt)
                else:
                    # output-major: k and two q heads on the psA ring, the
                    # other two q heads in halves of one psS pair tile
                    sp = psS.tile([P, 2, TCH], F32, tag="s", name="accp")
                    accs = {}
                    for i, o in enumerate((4, 0, 1, 2, 3)):
                        if o in (2, 3):
                            accs[o] = sp[:, o - 2, :]
                        else:
                            accs[o] = psA.tile(
                                [P, TCH], F32, tag="acc", name=f"acc{o}")
                        for k in range(NK):
                            nc.tensor.matmul(
                                accs[o][:], w_slice(o, k), xs[:, k, :],
                                start=(k == 0), stop=(k == NK - 1))
                    qk_finish(accs, jt)

                if jt + 1 < NJT:
                    xch_cur = load_x(jt + 1)

                # v: x chunk stationary, wv moving
                for tl in range(TCH // P):
                    tt = jt * (TCH // P) + tl
                    vt_ps = psA.tile([P, D], F32, tag="acc", name="vt")
                    for k in range(NK):
                        nc.tensor.matmul(
                            vt_ps[:], xs[:, k, tl * P:(tl + 1) * P],
                            wv_sb[:, k, :],
                            start=(k == 0), stop=(k == NK - 1))
                    nc.scalar.activation(v_sb[:, tt, :], vt_ps[:], AF.Copy)

            # ---- B(jt): attention; C(jt-1) interleaved per head -------
            if "B" in phases:
                if jt == 0:
                    for jc in range(NJT):
                        nc.scalar.dma_start(
                            wo_sb[:, :, jc * TCH:(jc + 1) * TCH],
                            wo_d[:, jc * TCH:(jc + 1) * TCH].rearrange(
                                "(h p) c -> p h c", p=P))
                njs = 4 * jt + 4
                for h in range(NH):
                    ot_ps = psO.tile([P, TCH], F32, tag="ot")
                    qch = qrot[:, h, ch]
                    dn = dnp.tile([P, TCH], sb_dt, tag="dn")
                    cq = list(range(NJT)) if jt > 0 else []  # pending C jc's

                    def emit_pv(es, js, half, toff, njs=njs, ot_ps=ot_ps):
                        nc.tensor.matmul(
                            ot_ps[:, toff:], v_sb[:, js, :],
                            es[:, half, toff:],
                            start=(js == 0), stop=(js == njs - 1),
                            skip_group_check=True)

                    pend = []  # deferred so exp latency is hidden
                    for jp in range(njs // 2):
                        sp = psS.tile([P, 2, TCH], F32, tag="s")
                        halves = []
                        for half in (0, 1):
                            js = 2 * jp + half
                            toff = P * (js - 4 * jt) if js >= 4 * jt else 0
                            nc.tensor.matmul(
                                sp[:, half, toff:],
                                krot[:, js * P:(js + 1) * P], qch[:, toff:],
                                start=True, stop=True, skip_group_check=True)
                            halves.append((js, toff))
                        es = esp.tile([P, 2, TCH], sb_dt, tag="es")
                        if halves[0][1] == 0 and halves[1][1] == 0:
                            nc.scalar.activation(
                                es[:, :, :], sp[:, :, :], AF.Exp, scale=SCALE)
                        else:  # diagonal halves: exp only the computed part
                            for half, (js, toff) in enumerate(halves):
                                nc.scalar.activation(
                                    es[:, half, toff:], sp[:, half, toff:],
                                    AF.Exp, scale=SCALE)
                        for half, (js, toff) in enumerate(halves):
                            if js >= 4 * jt:  # diagonal: triangular mask
                                nc.vector.tensor_tensor(
                                    es[:, half, toff:toff + P],
                                    es[:, half, toff:toff + P],
                                    tri_sb[:], ALU.mult)
                            if js == 0:
                                nc.vector.tensor_copy(dn[:], es[:, half, :])
                            else:
                                nc.vector.tensor_tensor(
                                    dn[:, toff:], dn[:, toff:],
                                    es[:, half, toff:], ALU.add)
                            if len(pend) >= 4:
                                emit_pv(*pend.pop(0))
                            pend.append((es, js, half, toff))
                        if cq:  # fill with prev-chunk out-proj
                            emit_C1(jt - 1, h, cq.pop(0))
                    for p_ in pend:
                        emit_pv(*p_)
                    for jc in cq:
                        emit_C1(jt - 1, h, jc)

                    den = dnp.tile([P, TCH], sb_dt, tag="den")
                    rec = dnp.tile([P, TCH], sb_dt, tag="rec")
                    if jt == NJT - 1 and h == NH - 1:
                        # final head gates the last out-proj block: run the
                        # reduce/normalize chain in 128-col pieces so the
                        # first C(3) group unblocks early
                        for q in range(4):
                            sl_ = slice(q * P, (q + 1) * P)
                            chq = slice(jt * TCH + q * P,
                                        jt * TCH + (q + 1) * P)
                            nc.gpsimd.partition_all_reduce(
                                den[:, sl_], dn[:, sl_], 128,
                                bass_isa.ReduceOp.add)
                            with nc.allow_low_precision(reason="f16 denom"):
                                nc.vector.reciprocal(
                                    rec[:, sl_], den[:, sl_])
                            nc.vector.tensor_tensor(
                                ot_sb[:, h, chq], ot_ps[:, sl_],
                                rec[:, sl_], ALU.mult)
                    else:
                        nc.gpsimd.partition_all_reduce(
                            den[:], dn[:], 128, bass_isa.ReduceOp.add)
                        with nc.allow_low_precision(reason="f16 denom"):
                            nc.vector.reciprocal(rec[:], den[:])
                        nc.vector.tensor_tensor(
                            ot_sb[:, h, ch], ot_ps[:], rec[:], ALU.mult)


        # ---- C(3): final chunk's output projection --------------------
        if "C" in phases:
            for tl in range(TCH // P):
                for jc in range(NJT):
                    emit_C1(NJT - 1, tl, jc)

    nc.compile()
    return nc


def host_prep(x, wq, wk, wv, wo, mode="f16"):
    """Build the 8 per-core input maps (numpy, host-side reshuffles only)."""
    ndt = _np_dt(mode)
    x = np.asarray(x, dtype=np.float32)
    wq = np.asarray(wq, dtype=np.float32)
    wk = np.asarray(wk, dtype=np.float32)
    wv = np.asarray(wv, dtype=np.float32)
    wo = np.asarray(wo, dtype=np.float32)

    # RoPE even/odd grouping permutation within each head
    perm = np.concatenate([np.arange(0, D, 2), np.arange(1, D, 2)])

    # rope tables, transposed layout [d, t]
    inv_freq = (1.0 / THETA ** (np.arange(0, D, 2, dtype=np.float32) / D)
                ).astype(np.float32)
    pos = np.arange(T, dtype=np.float32)
    freqs = pos[:, None] * inv_freq[None, :]          # [T, 64] f32
    cos_t = np.cos(freqs).astype(np.float32).T        # [64, T]
    sin_t = np.sin(freqs).astype(np.float32).T        # [64, T]
    cosT = np.concatenate([cos_t, cos_t], axis=0)     # [128, T]
    sinT = np.concatenate([-sin_t, sin_t], axis=0)    # [128, T]

    # triangular multiplicative mask for diagonal blocks: allow f >= p
    f = np.arange(P)[None, :]
    p = np.arange(P)[:, None]
    tri = (f >= p).astype(np.float32)

    xTs = [np.ascontiguousarray(x[b].T).astype(ndt) for b in range(B)]

    in_maps = []
    for c in range(N_CORES):
        b, g = divmod(c, GROUP)
        rows = []
        for hh in range(NH):
            h = g * GROUP + hh
            rows.append(wq[h * D + perm, :])
        wq_g = np.concatenate(rows, axis=0)                  # [512, C]
        wk_g = wk[g * D + perm, :]                           # [128, C]
        wv_g = wv[g * D:(g + 1) * D, :]                      # [128, C]
        wo_g = wo[:, g * NH * D:(g + 1) * NH * D]            # [C, 512]

        in_maps.append({
            "xT": xTs[b],
            "wqT": np.ascontiguousarray(wq_g.T).astype(ndt),
            "wkT": np.ascontiguousarray(
                wk_g.T.reshape(NK, P, D).transpose(1, 0, 2)).astype(ndt),
            "wvT": np.ascontiguousarray(
                wv_g.T.reshape(NK, P, D).transpose(1, 0, 2)).astype(ndt),
            "woT": np.ascontiguousarray(wo_g.T).astype(ndt),
            "cosT": cosT.astype(ndt),
            "sinT": sinT.astype(ndt),
            "triT": tri.astype(ndt),
        })
    return in_maps


_CACHE = {}


def _get_program(mode):
    if mode not in _CACHE:
        _CACHE[mode] = build_program(mode)
    return _CACHE[mode]


def kernel(x, mask, wq, wk, wv, wo):
    mode = os.environ.get("BASS_ATTN_MODE", "f16")
    nc = _get_program(mode)
    in_maps = host_prep(x, wq, wk, wv, wo, mode)
    res = run_bass_kernel_spmd(nc, in_maps, list(range(N_CORES))).results
    out = np.zeros((B, T, C), dtype=np.float32)
    for c in range(N_CORES):
        out[c // GROUP] += res[c]["y"]
    return out
